# revision 1
# baseline (speedup 1.0000x reference)
"""Trainium2 Bass kernel for FConv2d (FFT conv module).

out = irfftn( rfftn(x, axes=(c,h,w)) * rfftn(pad(weight)) )[:, :, ::4] reshaped.

Strategy (data-parallel over batch, 4 per core x 8 cores):
  S1  channel DFT (c=128) as matmuls; also transposes hw chunks onto partitions
  S2  joint 2D spatial DFT (contract hw=1024 via PSUM-accumulated matmuls);
      the complex combine is folded into the accumulation via a negated X1i
  MUL elementwise complex multiply with host-precomputed folded W-hat   (DVE)
  I1  fold(128->32) + subsampled channel IDFT via paired complex matmuls
  I2  joint 2D spatial inverse + real-part extraction via paired matmuls

All matmuls run as float32r (full-rate fp32 mode on TRN2 PE).
Constants (DFT factor matrices, W-hat) are computed host-side in numpy and
fed as replicated ExternalInputs.
"""
import sys
import numpy as np

for _p in ("/opt/trn_rl_repo", "/root/.axon_site/_ro/trn_rl_repo"):
    if _p not in sys.path:
        sys.path.insert(0, _p)

import concourse.bacc as bacc
import concourse.bass as bass
import concourse.mybir as mybir
import concourse.tile as tile
from concourse.bass_utils import run_bass_kernel_spmd

F32 = mybir.dt.float32
F32R = mybir.dt.float32r

B = 32          # full batch
B_LOC = 4       # per core
N_CORES = 8
CIN = 128
L = 32
NFIL = 8        # num filters n
NF = 544        # stored spatial freqs (32 * 17)
NFP = 640       # padded: 5 chunks of 128


# ----------------------------------------------------------------- constants
def build_constants(weight):
    c = np.arange(128)
    k = np.arange(128)
    ang = 2 * np.pi * np.outer(c, k) / 128.0
    fc_pack = np.concatenate([np.cos(ang), -np.sin(ang)], axis=1).astype(np.float32)

    f = np.arange(NFP)
    p = np.where(f < NF, f // 17, 0)
    q = np.where(f < NF, f % 17, 0)
    valid = (f < NF).astype(np.float64)

    # f2d [128, 8, 5, 384]: cols of block fb: [-sin | cos | +sin].
    # With PSUM blocks laid out [Xi | Xr], the A-pass (lhsT=X1r) reads
    # [-sin | cos] (cols 0:256) and the B-pass (lhsT=X1i) reads
    # [cos | sin] (cols 128:384) -- both contiguous, N=256 per block, so
    # fb-paired matmuls run at N=512 full fp32r rate.
    f2d = np.zeros((128, 8, 5, 384), dtype=np.float32)
    hw_p = np.arange(128)
    for t in range(8):
        h = 4 * t + hw_p // 32
        w = hw_p % 32
        for fb in range(5):
            sl = slice(fb * 128, (fb + 1) * 128)
            ang2 = 2 * np.pi * (np.outer(h, p[sl]) + np.outer(w, q[sl])) / 32.0
            f2d[:, t, fb, 0:128] = -np.sin(ang2) * valid[sl]
            f2d[:, t, fb, 128:256] = np.cos(ang2) * valid[sl]
            f2d[:, t, fb, 256:384] = np.sin(ang2) * valid[sl]

    kk = np.arange(128)
    j = np.arange(32)
    ang3 = 2 * np.pi * np.outer(kk, j) / 32.0
    er = np.cos(ang3)
    ei = np.sin(ang3)
    epack1 = np.concatenate([er, ei], axis=1).astype(np.float32)
    epack2 = np.concatenate([-ei, er], axis=1).astype(np.float32)

    k2d = np.zeros((128, 5, 2, 1024), dtype=np.float32)
    yz = np.arange(1024)
    y = yz // 32
    z = yz % 32
    for fc in range(5):
        sl = slice(fc * 128, (fc + 1) * 128)
        ang4 = 2 * np.pi * (np.outer(p[sl], y) + np.outer(q[sl], z)) / 32.0
        k2d[:, fc, 0, :] = np.cos(ang4) * valid[sl][:, None]
        k2d[:, fc, 1, :] = -np.sin(ang4) * valid[sl][:, None]

    w_hat = np.fft.rfftn(weight.astype(np.float64), s=(CIN, L, L), axes=(1, 2, 3))
    alpha = np.full(17, 2.0)
    alpha[0] = 1.0
    alpha[16] = 1.0
    w_hat = w_hat * alpha[None, None, None, :] / (128.0 * 32.0 * 32.0)
    wr = np.zeros((128, NFIL, NFP), dtype=np.float32)
    wi = np.zeros((128, NFIL, NFP), dtype=np.float32)
    wr[:, :, :NF] = np.transpose(w_hat.real, (1, 0, 2, 3)).reshape(128, NFIL, NF)
    wi[:, :, :NF] = np.transpose(w_hat.imag, (1, 0, 2, 3)).reshape(128, NFIL, NF)
    return {
        "fc_pack": fc_pack,
        "f2d": f2d,
        "epack1": epack1,
        "epack2": epack2,
        "k2d": k2d,
        "wr": wr,
        "wi": wi,
    }


# ----------------------------------------------------------------- program
def build_program(dbg=False):
    nc = bacc.Bacc("TRN2", target_bir_lowering=False, debug=False)
    x_d = nc.dram_tensor("x", [B_LOC, 128, 32, 32], F32R, kind="ExternalInput")
    fc_d = nc.dram_tensor("fc_pack", [128, 256], F32R, kind="ExternalInput")
    f2d_d = nc.dram_tensor("f2d", [128, 8, 5, 384], F32R, kind="ExternalInput")
    e1_d = nc.dram_tensor("epack1", [128, 64], F32R, kind="ExternalInput")
    e2_d = nc.dram_tensor("epack2", [128, 64], F32R, kind="ExternalInput")
    k2d_d = nc.dram_tensor("k2d", [128, 5, 2, 1024], F32R, kind="ExternalInput")
    wr_d = nc.dram_tensor("wr", [128, NFIL, NFP], F32R, kind="ExternalInput")
    wi_d = nc.dram_tensor("wi", [128, NFIL, NFP], F32R, kind="ExternalInput")
    out_d = nc.dram_tensor("out", [B_LOC, 256, 32, 32], F32, kind="ExternalOutput")
    if dbg:
        dbg_x1 = nc.dram_tensor("dbg_x1", [128, 8, 2, 128], F32R, kind="ExternalOutput")
        dbg_xf = nc.dram_tensor("dbg_xf", [128, 2, NFP], F32R, kind="ExternalOutput")
        dbg_a = nc.dram_tensor("dbg_a", [128, 5, 2, NFIL, 32], F32R, kind="ExternalOutput")

    with tile.TileContext(nc) as tc:
        with (
            tc.tile_pool(name="consts", bufs=1) as cpool,
            tc.tile_pool(name="xin", bufs=2) as xpool,
            tc.tile_pool(name="x1", bufs=2) as x1pool,
            tc.tile_pool(name="xf", bufs=2) as xfpool,
            tc.tile_pool(name="z", bufs=2) as zpool,
            tc.tile_pool(name="a", bufs=1) as apool,
            tc.tile_pool(name="o", bufs=3) as opool,
            tc.tile_pool(name="ps1", bufs=2, space="PSUM") as ps1pool,
            tc.tile_pool(name="ps2", bufs=3, space="PSUM") as ps2pool,
            tc.tile_pool(name="psi1", bufs=1, space="PSUM") as psi1pool,
            tc.tile_pool(name="psi2", bufs=2, space="PSUM") as psi2pool,
        ):
            # ---- load constants; ordered + chunked so early compute
            # stages never wait on late-needed constants.
            fc_sb = cpool.tile([128, 256], F32R)
            nc.sync.dma_start(out=fc_sb[:], in_=fc_d[:])
            e1_sb = cpool.tile([128, 64], F32R)
            nc.sync.dma_start(out=e1_sb[:], in_=e1_d[:])
            e2_sb = cpool.tile([128, 64], F32R)
            nc.sync.dma_start(out=e2_sb[:], in_=e2_d[:])
            f2d_sb = cpool.tile([128, 8, 5, 384], F32R)
            for t in range(8):
                nc.sync.dma_start(out=f2d_sb[:, t], in_=f2d_d[:, t])
            wr_sb = cpool.tile([128, NFIL, NFP], F32R)
            nc.sync.dma_start(out=wr_sb[:], in_=wr_d[:])
            wi_sb = cpool.tile([128, NFIL, NFP], F32R)
            nc.sync.dma_start(out=wi_sb[:], in_=wi_d[:])
            k2d_sb = cpool.tile([128, 5, 2, 1024], F32R)
            for fc in range(5):
                nc.sync.dma_start(out=k2d_sb[:, fc], in_=k2d_d[:, fc])

            for b in range(B_LOC):
                # ---- load x[b]: [c=128, h, w]
                xt = xpool.tile([128, 32, 32], F32R, tag="xt")
                nc.scalar.dma_start(out=xt[:], in_=x_d[b])

                # ---- S1: channel DFT; x1 slots: 0=X1r, 1=X1i
                x1 = x1pool.tile([128, 8, 2, 128], F32R, tag="x1")
                for t in range(8):
                    ps = ps1pool.tile([128, 256], F32, tag="ps1")
                    nc.tensor.matmul(
                        ps[:], xt[:, 4 * t:4 * t + 4, :], fc_sb[:],
                        start=True, stop=True,
                    )
                    nc.scalar.copy(x1[:, t, 0, :], ps[:, 0:128])
                    nc.scalar.copy(x1[:, t, 1, :], ps[:, 128:256])

                if dbg and b == 0:
                    nc.sync.dma_start(out=dbg_x1[:], in_=x1[:])

                # ---- S2: joint 2D DFT, complex combine via PSUM accumulation.
                # psum block fb: cols [Xr(128) | Xi(128)]
                # Xr = X1r@cos + X1i@sin ;  Xi = X1i@cos - X1r@sin
                s2ps = [
                    ps2pool.tile([128, 512], F32, tag="ps2", name=f"s2ps{b}_{g}")
                    for g in range(3)
                ]

                def s2slot(fb):
                    return s2ps[fb // 2][:, (fb % 2) * 256:(fb % 2) * 256 + 256]

                # One accumulation group per PSUM bank: the A-pass spans the
                # whole bank (fb-pair) and carries the only start=True.
                # PSUM block fb = [Xi(128) | Xr(128)]:
                #   A (lhsT=X1r) @ [-sin | cos] ;  B (lhsT=X1i) @ [cos | sin]
                for t in range(8):
                    for g in range(3):
                        slot = s2ps[g]
                        width = 512 if g < 2 else 256
                        pair = slice(2 * g, 2 * g + 2) if g < 2 else slice(4, 5)
                        nc.tensor.matmul(
                            slot[:, 0:width], x1[:, t, 0, :],
                            f2d_sb[:, t, pair, 0:256],
                            start=(t == 0), stop=False,
                            skip_group_check=True,
                        )
                    for g in range(3):
                        slot = s2ps[g]
                        width = 512 if g < 2 else 256
                        pair = slice(2 * g, 2 * g + 2) if g < 2 else slice(4, 5)
                        nc.tensor.matmul(
                            slot[:, 0:width], x1[:, t, 1, :],
                            f2d_sb[:, t, pair, 128:384],
                            start=False, stop=(t == 7),
                            skip_group_check=True,
                        )

                # copy PSUM -> SBUF, de-interleaving to contiguous
                # xr_c / xi_c [128, 640] (f-major) for fast 1D DVE ops
                xr_c = xfpool.tile([128, NFP], F32R, tag="xr")
                xi_c = xfpool.tile([128, NFP], F32R, tag="xi")
                for g in range(3):
                    slot = s2ps[g]
                    if g < 2:
                        sv = slot[:].rearrange("p (a c) -> p a c", a=2)
                        nc.scalar.copy(
                            xi_c[:, g * 256:(g + 1) * 256], sv[:, :, 0:128])
                        nc.scalar.copy(
                            xr_c[:, g * 256:(g + 1) * 256], sv[:, :, 128:256])
                    else:
                        nc.scalar.copy(xi_c[:, 512:640], slot[:, 0:128])
                        nc.scalar.copy(xr_c[:, 512:640], slot[:, 128:256])

                if dbg and b == 0:
                    nc.sync.dma_start(out=dbg_xf[:, 0], in_=xr_c[:])
                    nc.sync.dma_start(out=dbg_xf[:, 1], in_=xi_c[:])

                # ---- per-n: complex multiply (DVE) + I1 matmuls
                a_sb = apool.tile([128, 5, 2, NFIL, 32], F32R, tag="a")
                for n in range(8):
                    zr = zpool.tile([128, NFP], F32R, tag="zr")
                    zi = zpool.tile([128, NFP], F32R, tag="zi")
                    t1 = zpool.tile([128, NFP], F32R, tag="t1", bufs=1)
                    t2 = zpool.tile([128, NFP], F32R, tag="t2", bufs=1)
                    # only the 544 real freqs on DVE; pads zeroed by idle
                    # GpSimd so I1's last-chunk lhsT reads clean zeros
                    nc.gpsimd.memset(zr[:, NF:NFP].bitcast(F32), 0.0)
                    nc.gpsimd.memset(zi[:, NF:NFP].bitcast(F32), 0.0)
                    nc.vector.tensor_mul(t1[:, 0:NF], xr_c[:, 0:NF], wr_sb[:, n, 0:NF])
                    nc.vector.tensor_mul(t2[:, 0:NF], xi_c[:, 0:NF], wi_sb[:, n, 0:NF])
                    nc.vector.tensor_sub(zr[:, 0:NF], t1[:, 0:NF], t2[:, 0:NF])
                    nc.vector.tensor_mul(t1[:, 0:NF], xr_c[:, 0:NF], wi_sb[:, n, 0:NF])
                    nc.vector.tensor_mul(t2[:, 0:NF], xi_c[:, 0:NF], wr_sb[:, n, 0:NF])
                    nc.vector.tensor_add(zi[:, 0:NF], t1[:, 0:NF], t2[:, 0:NF])

                    ips = psi1pool.tile([128, 320], F32, tag="psi1")
                    for fc in range(5):
                        col = slice(fc * 64, (fc + 1) * 64)
                        zsl = slice(fc * 128, (fc + 1) * 128)
                        nc.tensor.matmul(
                            ips[:, col], zr[:, zsl], e1_sb[:],
                            start=True, stop=False,
                        )
                        nc.tensor.matmul(
                            ips[:, col], zi[:, zsl], e2_sb[:],
                            start=False, stop=True,
                        )
                    # scatter [fc, comp, j] cols of ips into a_sb[:, fc, comp, n, :]
                    nc.scalar.copy(
                        a_sb[:, :, :, n, :],
                        ips[:].rearrange("p (fc c j) -> p fc c j", fc=5, c=2),
                    )

                if dbg and b == 0:
                    nc.sync.dma_start(out=dbg_a[:], in_=a_sb[:])

                # ---- I2: joint 2D inverse + Re extraction
                for mh in range(2):
                    for nzc in range(2):
                        ops = psi2pool.tile([128, 512], F32, tag="psi2")
                        for fc in range(5):
                            for comp in range(2):
                                lhsT = a_sb[:, fc, comp,
                                            mh * 4:(mh + 1) * 4, :]
                                rhs = k2d_sb[:, fc, comp,
                                             nzc * 512:(nzc + 1) * 512]
                                nc.tensor.matmul(
                                    ops[:], lhsT, rhs,
                                    start=(fc == 0 and comp == 0),
                                    stop=(fc == 4 and comp == 1),
                                )
                        o_sb = opool.tile([128, 512], F32, tag="o")
                        nc.scalar.copy(o_sb[:], ops[:])
                        dst = out_d[b, mh * 128:(mh + 1) * 128].rearrange(
                            "c h w -> c (h w)")[:, nzc * 512:(nzc + 1) * 512]
                        nc.scalar.dma_start(out=dst, in_=o_sb[:])
    nc.compile()
    return nc


_CACHE = {}


def kernel(x, weight):
    x = np.ascontiguousarray(np.asarray(x, dtype=np.float32))
    weight = np.asarray(weight, dtype=np.float32)
    consts = build_constants(weight)
    if "nc" not in _CACHE:
        _CACHE["nc"] = build_program()
    nc = _CACHE["nc"]
    in_maps = []
    for i in range(N_CORES):
        m = {"x": x[i * B_LOC:(i + 1) * B_LOC]}
        m.update(consts)
        in_maps.append(m)
    res = run_bass_kernel_spmd(nc, in_maps, core_ids=list(range(N_CORES)))
    out = np.concatenate([r["out"] for r in res.results], axis=0)
    return out


if __name__ == "__main__":
    import jax

    sys.path.insert(0, "/root/problem")
    from reference import setup_inputs, reference

    with jax.default_device(jax.devices("cpu")[0]):
        inputs = setup_inputs()
        inputs = {k: np.asarray(v) for k, v in inputs.items()}
        expected = np.asarray(reference(**inputs))
    actual = kernel(**inputs)
    err = np.linalg.norm(actual - expected) / np.linalg.norm(expected)
    print("Relative error:", err)



# revision 17
# speedup vs baseline: 2.4387x; 2.4387x over previous
"""Trainium2 Bass kernel for FConv2d (FFT conv module), v2.

out = irfftn( rfftn(x, axes=(c,h,w)) * rfftn(pad(weight)) )[:, :, ::4] reshaped.

Data-parallel over batch: 4 elements per core x 8 cores. Per element:
  S1   channel DFT (contract c=128) as 8 matmuls -> X1 [hw | (kr|ki)]
  S2a  separable w-DFT (contract w inside hw-chunk partitions, block-diag
       over the 4 h's of the chunk) -> Y [k | (c,q,h-parts)]
  T    PE transposes -> YT [(qs,h) | k] per (qc, comp)
  S2b  separable h-DFT (contract (h,qsub) per q-chunk) -> X [k | f], with
       f enumerated (qc, p, qsub)
  MUL  Gauss 3-mult complex multiply on DVE/Pool with host-folded
       w1=wr, w2=wi-wr, w3=wi+wr  (k1=w1*(xr+xi), k2=xr*w2, k3=xi*w3)
  I1   subsampled channel IDFT, 3 passes (E1=e1+e2, E2=e2, E3=-e1) folding
       the Gauss combine into PSUM accumulation -> A [f | (c,j)]
  I2   joint 2D spatial inverse + Re extraction (10 accumulated matmuls
       per output quarter)
All matmuls/transposes run in bf16 (full-rate on the TRN2 PE); PSUM
accumulates in f32.  Constants are ~6.3MB bf16 (vs 18.6MB f32 before).
"""
import sys
import numpy as np
import ml_dtypes

for _p in ("/opt/trn_rl_repo", "/root/.axon_site/_ro/trn_rl_repo"):
    if _p not in sys.path:
        sys.path.insert(0, _p)

import concourse.bacc as bacc
import concourse.mybir as mybir
import concourse.tile as tile
from concourse.bass_utils import run_bass_kernel_spmd

F32 = mybir.dt.float32
BF16 = mybir.dt.bfloat16
NPBF = ml_dtypes.bfloat16

B = 32
B_LOC = 4
N_CORES = 8
CIN = 128
L = 32
NFIL = 8
NF = 544

# csmall column offsets
_O_FC = 0          # [128, 256]
_O_WDA = 256       # [128, 136]
_O_WDB = 392       # [128, 136]
_O_HDA = 528       # [128, 256]
_O_HDB = 784       # [128, 256]
_O_H4A = 1040      # [32, 64]
_O_H4B = 1104      # [32, 64]
_O_E1 = 1168       # [128, 64]
_O_E2 = 1232       # [128, 64]
_O_E3 = 1296       # [128, 64]
_O_ID = 1360       # [128, 128]
_CSMALL = 1488


def _f_index():
    ps, qs = [], []
    for qc in range(5):
        w = 4 if qc < 4 else 1
        for p in range(32):
            for s in range(w):
                ps.append(p)
                qs.append(qc * 4 + s)
    return np.array(ps), np.array(qs)


def build_constants(weight):
    csmall = np.zeros((128, _CSMALL), dtype=np.float32)

    c = np.arange(128)
    k = np.arange(128)
    ang = 2 * np.pi * np.outer(c, k) / 128.0
    csmall[:, _O_FC:_O_FC + 128] = np.cos(ang)
    csmall[:, _O_FC + 128:_O_FC + 256] = -np.sin(ang)

    # S2a: wdftA/B [hw=(hl,w) | (hl,q,c)=136], block-diag over hl
    hl = np.arange(128) // 32
    w = np.arange(128) % 32
    q = np.arange(17)
    wdA = np.zeros((128, 4, 17, 2), dtype=np.float32)
    wdB = np.zeros((128, 4, 17, 2), dtype=np.float32)
    for r in range(128):
        angw = 2 * np.pi * w[r] * q / 32.0
        wdA[r, hl[r], :, 0] = np.cos(angw)
        wdA[r, hl[r], :, 1] = -np.sin(angw)
        wdB[r, hl[r], :, 0] = np.sin(angw)
        wdB[r, hl[r], :, 1] = np.cos(angw)
    csmall[:, _O_WDA:_O_WDA + 136] = wdA.reshape(128, 136)
    csmall[:, _O_WDB:_O_WDB + 136] = wdB.reshape(128, 136)

    # S2b: hdftA/B rows=(qs,h): r = qs*32+h; cols (c, p_out, qs')=256
    hdA = np.zeros((128, 2, 32, 4), dtype=np.float32)
    hdB = np.zeros((128, 2, 32, 4), dtype=np.float32)
    for r in range(128):
        s, h = r // 32, r % 32
        angh = 2 * np.pi * h * np.arange(32) / 32.0
        hdA[r, 0, :, s] = np.cos(angh)
        hdA[r, 1, :, s] = -np.sin(angh)
        hdB[r, 0, :, s] = np.sin(angh)
        hdB[r, 1, :, s] = np.cos(angh)
    csmall[:, _O_HDA:_O_HDA + 256] = hdA.reshape(128, 256)
    csmall[:, _O_HDB:_O_HDB + 256] = hdB.reshape(128, 256)

    # qc=4 (q=16): rows = h (32); cols (c, p_out)=64
    h4A = np.zeros((32, 2, 32), dtype=np.float32)
    h4B = np.zeros((32, 2, 32), dtype=np.float32)
    for h in range(32):
        angh = 2 * np.pi * h * np.arange(32) / 32.0
        h4A[h, 0] = np.cos(angh)
        h4A[h, 1] = -np.sin(angh)
        h4B[h, 0] = np.sin(angh)
        h4B[h, 1] = np.cos(angh)
    csmall[0:32, _O_H4A:_O_H4A + 64] = h4A.reshape(32, 64)
    csmall[0:32, _O_H4B:_O_H4B + 64] = h4B.reshape(32, 64)

    # I1 rhs: E1/E2/E3 [k | (c,j)=64]
    kk = np.arange(128)
    jj = np.arange(32)
    ang3 = 2 * np.pi * np.outer(kk, jj) / 32.0
    er, ei = np.cos(ang3), np.sin(ang3)
    e1 = np.concatenate([er, ei], axis=1)
    e2 = np.concatenate([-ei, er], axis=1)
    csmall[:, _O_E1:_O_E1 + 64] = e1 + e2
    csmall[:, _O_E2:_O_E2 + 64] = e2
    csmall[:, _O_E3:_O_E3 + 64] = -e1

    csmall[:, _O_ID:_O_ID + 128] = np.eye(128, dtype=np.float32)

    # W packs [k | pack, n, f]
    p_of_f, q_of_f = _f_index()
    w_hat = np.fft.rfftn(weight.astype(np.float64), s=(CIN, L, L), axes=(1, 2, 3))
    alpha = np.full(17, 2.0)
    alpha[0] = 1.0
    alpha[16] = 1.0
    w_hat = w_hat * alpha[None, None, None, :] / (128.0 * 32.0 * 32.0)
    whf = w_hat[:, :, p_of_f, q_of_f]            # [n, k, f]
    wr = np.transpose(whf.real, (1, 0, 2)).astype(np.float32)   # [k, n, f]
    wi = np.transpose(whf.imag, (1, 0, 2)).astype(np.float32)
    wpack = np.stack([wr, wi - wr, wi + wr], axis=2)  # [k, n, 3, f]

    # I2 rhs: k2d [k-rows | qc, c, yz]
    yz = np.arange(1024)
    y, z = yz // 32, yz % 32
    k2d = np.zeros((128, 5, 2, 1024), dtype=np.float32)
    for qc in range(5):
        rows = 128 if qc < 4 else 32
        fbase = qc * 128
        for r in range(rows):
            p_, q_ = p_of_f[fbase + r], q_of_f[fbase + r]
            ang4 = 2 * np.pi * (p_ * y + q_ * z) / 32.0
            k2d[r, qc, 0] = np.cos(ang4)
            k2d[r, qc, 1] = -np.sin(ang4)

    return {
        "csmall": csmall.astype(NPBF),
        "wpack": wpack.astype(NPBF),
        "k2d": k2d.astype(NPBF),
    }


def build_program(dbg=False):
    nc = bacc.Bacc("TRN2", target_bir_lowering=False, debug=False)
    x_d = nc.dram_tensor("x", [B_LOC, 128, 1024], BF16, kind="ExternalInput")
    cs_d = nc.dram_tensor("csmall", [128, _CSMALL], BF16, kind="ExternalInput")
    wp_d = nc.dram_tensor("wpack", [128, NFIL, 3, NF], BF16, kind="ExternalInput")
    k2_d = nc.dram_tensor("k2d", [128, 5, 2, 1024], BF16, kind="ExternalInput")
    out_d = nc.dram_tensor("out", [B_LOC, 256, 1024], F32, kind="ExternalOutput")

    with tile.TileContext(nc) as tc:
        with (
            tc.tile_pool(name="consts", bufs=1) as cpool,
            tc.tile_pool(name="xin", bufs=3) as xpool,
            tc.tile_pool(name="x1", bufs=2) as x1pool,
            tc.tile_pool(name="y", bufs=2) as ypool,
            tc.tile_pool(name="yt", bufs=2) as ytpool,
            tc.tile_pool(name="xf", bufs=4) as xfpool,
            tc.tile_pool(name="z", bufs=6) as zpool,
            tc.tile_pool(name="a", bufs=3) as apool,
            tc.tile_pool(name="o", bufs=2) as opool,
            tc.tile_pool(name="fw", bufs=2, space="PSUM") as fwpool,
            tc.tile_pool(name="ip", bufs=2, space="PSUM") as ippool,
        ):
            cs = cpool.tile([128, _CSMALL], BF16)
            nc.scalar.dma_start(out=cs[:], in_=cs_d[:])
            wp = cpool.tile([128, NFIL, 3, NF], BF16)
            k2 = cpool.tile([128, 5, 2, 1024], BF16)
            nc.scalar.dma_start(out=wp[:, 0:2], in_=wp_d[:, 0:2])
            nc.scalar.dma_start(out=wp[:, 2:4], in_=wp_d[:, 2:4])
            nc.scalar.dma_start(out=k2[:, 0:2], in_=k2_d[:, 0:2])
            nc.scalar.dma_start(out=wp[:, 4:6], in_=wp_d[:, 4:6])
            nc.scalar.dma_start(out=k2[:, 2:5], in_=k2_d[:, 2:5])
            nc.scalar.dma_start(out=wp[:, 6:8], in_=wp_d[:, 6:8])

            fc = cs[:, _O_FC:_O_FC + 256]
            wdA = cs[:, _O_WDA:_O_WDA + 136]
            wdB = cs[:, _O_WDB:_O_WDB + 136]
            hdA = cs[:, _O_HDA:_O_HDA + 256]
            hdB = cs[:, _O_HDB:_O_HDB + 256]
            h4A = cs[0:32, _O_H4A:_O_H4A + 64]
            h4B = cs[0:32, _O_H4B:_O_H4B + 64]
            E1 = cs[:, _O_E1:_O_E1 + 64]
            E2 = cs[:, _O_E2:_O_E2 + 64]
            E3 = cs[:, _O_E3:_O_E3 + 64]
            ident = cs[:, _O_ID:_O_ID + 128]

            state = {}

            def f_s1(b):
                xt = xpool.tile([128, 1024], BF16, tag="xt")
                (nc.sync if b < 2 else nc.scalar).dma_start(
                    out=xt[:], in_=x_d[b])
                x1 = x1pool.tile([128, 8, 2, 128], BF16, tag="x1")
                for half in range(2):
                    ps = fwpool.tile([128, 1024], F32, tag="fw")
                    for i in range(4):
                        t = 4 * half + i
                        nc.tensor.matmul(
                            ps[:, i * 256:(i + 1) * 256],
                            xt[:, t * 128:(t + 1) * 128], fc,
                            start=True, stop=True, skip_group_check=True,
                        )
                    nc.scalar.copy(
                        out=x1[:, 4 * half:4 * half + 2],
                        in_=ps[:, 0:512].rearrange(
                            "p (t c k) -> p t c k", t=2, c=2),
                    )
                    nc.vector.tensor_copy(
                        out=x1[:, 4 * half + 2:4 * half + 4],
                        in_=ps[:, 512:1024].rearrange(
                            "p (t c k) -> p t c k", t=2, c=2),
                    )
                state[("x1", b)] = x1

            def f_s2a(b):
                x1 = state.pop(("x1", b))
                Y = ypool.tile([128, 2, 17, 8, 4], BF16, tag="Y")

                def s2a_mm(ps, t, o):
                    nc.tensor.matmul(
                        ps[:, o:o + 136], x1[:, t, 0], wdA,
                        start=True, stop=False, skip_group_check=True,
                    )
                    nc.tensor.matmul(
                        ps[:, o:o + 136], x1[:, t, 1], wdB,
                        start=False, stop=True, skip_group_check=True,
                    )

                ps = fwpool.tile([128, 1024], F32, tag="fw")
                for i in range(3):
                    s2a_mm(ps, i, 136 * i)
                    s2a_mm(ps, 3 + i, 512 + 136 * i)
                nc.scalar.copy(
                    out=Y[:, :, :, 0:3, :].rearrange("p c q t hl -> p t hl q c"),
                    in_=ps[:, 0:408].rearrange(
                        "p (t hl q c) -> p t hl q c", t=3, hl=4, q=17),
                )
                nc.vector.tensor_copy(
                    out=Y[:, :, :, 3:6, :].rearrange("p c q t hl -> p t hl q c"),
                    in_=ps[:, 512:920].rearrange(
                        "p (t hl q c) -> p t hl q c", t=3, hl=4, q=17),
                )
                ps = fwpool.tile([128, 1024], F32, tag="fw")
                s2a_mm(ps, 6, 0)
                s2a_mm(ps, 7, 136)
                nc.scalar.copy(
                    out=Y[:, :, :, 6:8, :].rearrange("p c q t hl -> p t hl q c"),
                    in_=ps[:, 0:272].rearrange(
                        "p (t hl q c) -> p t hl q c", t=2, hl=4, q=17),
                )
                state[("Y", b)] = Y

            def f_t(b):
                Y = state.pop(("Y", b))
                yt = ytpool.tile([128, 10, 128], BF16, tag="yt")
                ps = fwpool.tile([128, 1024], F32, tag="fw")
                for c in range(2):
                    for qc in range(4):
                        slot = c * 4 + qc
                        nc.tensor.matmul(
                            ps[:, slot * 64:(slot + 1) * 64].bitcast(BF16),
                            Y[:, c, qc * 4:(qc + 1) * 4].rearrange(
                                "p q t hl -> p (q t hl)"),
                            ident, is_transpose=True,
                            start=True, stop=True, skip_group_check=True,
                        )
                for c in range(2):
                    nc.tensor.matmul(
                        ps[0:32, 512 + c * 64:512 + (c + 1) * 64].bitcast(BF16),
                        Y[:, c, 16].rearrange("p t hl -> p (t hl)"),
                        ident, is_transpose=True,
                        start=True, stop=True, skip_group_check=True,
                    )
                nc.scalar.copy(
                    out=yt[:, 0:5],
                    in_=ps[:, 0:320].bitcast(BF16).rearrange(
                        "p (s k) -> p s k", s=5),
                )
                nc.vector.tensor_copy(
                    out=yt[:, 5:10],
                    in_=ps[:, 320:640].bitcast(BF16).rearrange(
                        "p (s k) -> p s k", s=5),
                )
                state[("yt", b)] = yt

            def f_s2b(b):
                yt = state.pop(("yt", b))
                xr = xfpool.tile([128, NF], BF16, tag="xr")
                xi = xfpool.tile([128, NF], BF16, tag="xi")

                def s2b_mm(ps, qc, o):
                    nc.tensor.matmul(
                        ps[:, o:o + 256], yt[:, qc], hdA,
                        start=True, stop=False, skip_group_check=True,
                    )
                    nc.tensor.matmul(
                        ps[:, o:o + 256], yt[:, 4 + qc], hdB,
                        start=False, stop=True, skip_group_check=True,
                    )

                ps1 = fwpool.tile([128, 1024], F32, tag="fw")
                s2b_mm(ps1, 0, 0)
                s2b_mm(ps1, 1, 256)
                nc.tensor.matmul(
                    ps1[:, 512:576], yt[0:32, 8], h4A,
                    start=True, stop=False, skip_group_check=True,
                )
                nc.tensor.matmul(
                    ps1[:, 512:576], yt[0:32, 9], h4B,
                    start=False, stop=True, skip_group_check=True,
                )
                ps2 = fwpool.tile([128, 1024], F32, tag="fw")
                s2b_mm(ps2, 2, 0)
                s2b_mm(ps2, 3, 256)
                for psx, qg in ((ps1, 0), (ps2, 1)):
                    psv = psx[:, 0:512].rearrange(
                        "p (a c f) -> p a c f", a=2, c=2)
                    xrv = xr[:, qg * 256:(qg + 1) * 256].rearrange(
                        "p (a f) -> p a f", a=2)
                    xiv = xi[:, qg * 256:(qg + 1) * 256].rearrange(
                        "p (a f) -> p a f", a=2)
                    nc.vector.tensor_copy(out=xrv, in_=psv[:, :, 0])
                    nc.vector.tensor_copy(out=xiv, in_=psv[:, :, 1])
                ps4v = ps1[:, 512:576].rearrange("p (c f) -> p c f", c=2)
                nc.vector.tensor_copy(out=xr[:, 512:544], in_=ps4v[:, 0])
                nc.vector.tensor_copy(out=xi[:, 512:544], in_=ps4v[:, 1])
                s = xfpool.tile([128, NF], BF16, tag="s")
                nc.vector.tensor_add(s[:], xr[:], xi[:])
                a2 = apool.tile([128, 5, 2, NFIL, 32], BF16, tag="a2")
                o_sb = opool.tile([128, 2, 2, 512], F32, tag="o")
                state[b] = (xr, xi, s, a2, o_sb)

            def b_pair(b, np_):
                xr, xi, s, a2, o_sb = state[b]
                ips = ippool.tile([128, 1024], F32, tag="ip")
                for sub in range(2):
                    n = 2 * np_ + sub
                    base = 512 * sub
                    k1 = zpool.tile([128, NF], BF16, tag="k1")
                    k2t = zpool.tile([128, NF], BF16, tag="k2")
                    k3 = zpool.tile([128, NF], BF16, tag="k3")
                    nc.vector.tensor_mul(k2t[:], xr[:], wp[:, n, 1])
                    if n < 6:
                        nc.gpsimd.tensor_mul(k3[:], xi[:], wp[:, n, 2])
                    else:
                        nc.vector.tensor_mul(k3[:], xi[:], wp[:, n, 2])
                    nc.vector.tensor_mul(k1[:], wp[:, n, 0], s[:])
                    for qc in range(5):
                        rows = 128 if qc < 4 else 32
                        fs = slice(qc * 128, qc * 128 + rows)
                        o = base + qc * 64
                        nc.tensor.matmul(
                            ips[0:rows, o:o + 64], k1[:, fs], E1,
                            start=True, stop=False, skip_group_check=True,
                        )
                        nc.tensor.matmul(
                            ips[0:rows, o:o + 64], k2t[:, fs], E2,
                            start=False, stop=False, skip_group_check=True,
                        )
                        nc.tensor.matmul(
                            ips[0:rows, o:o + 64], k3[:, fs], E3,
                            start=False, stop=True, skip_group_check=True,
                        )
                eng = (nc.scalar, nc.vector)[np_ % 2]
                cp = eng.copy if eng is nc.scalar else eng.tensor_copy
                cp(
                    out=a2[:, :, :, 2 * np_:2 * np_ + 2, :].rearrange(
                        "p qc c n j -> p n qc c j"),
                    in_=ips[:].rearrange(
                        "p (pr qc c j) -> p pr qc c j", pr=2, qc=8, c=2)[
                        :, :, 0:5],
                )

            def b_i2(b, mh, nz):
                xr, xi, s, a2, o_sb = state[b]
                ops = ippool.tile([128, 1024], F32, tag="ip")
                for qc in range(5):
                    rows = 128 if qc < 4 else 32
                    for c in range(2):
                        lhsT = a2[0:rows, qc, c,
                                  mh * 4:(mh + 1) * 4, :].rearrange(
                            "p n j -> p (n j)")
                        rhs = k2[0:rows, qc, c, nz * 512:(nz + 1) * 512]
                        nc.tensor.matmul(
                            ops[:, 0:512], lhsT, rhs,
                            start=(qc == 0 and c == 0),
                            stop=(qc == 4 and c == 1),
                        )
                eng = (nc.scalar, nc.vector)[(mh * 2 + nz) % 2]
                cp = eng.copy if eng is nc.scalar else eng.tensor_copy
                cp(out=o_sb[:, mh, nz], in_=ops[:, 0:512])
                if nz == 1:
                    dst = out_d[b].rearrange(
                        "(mh p) (nz w) -> p mh nz w", mh=2, nz=2)[:, mh]
                    nc.sync.dma_start(out=dst, in_=o_sb[:, mh])
                    if mh == 1:
                        state.pop(b)

            # software-pipelined emission: interleave F(b+1) chunks with B(b)
            f_s1(0)
            f_s2a(0)
            f_t(0)
            f_s2b(0)
            for b in range(B_LOC):
                nb = b + 1
                has_f = nb < B_LOC
                if has_f:
                    f_s1(nb)
                b_pair(b, 0)
                if has_f:
                    f_s2a(nb)
                b_pair(b, 1)
                if has_f:
                    f_t(nb)
                b_i2(b, 0, 0)
                if has_f:
                    f_s2b(nb)
                b_i2(b, 0, 1)
                b_pair(b, 2)
                b_pair(b, 3)
                b_i2(b, 1, 0)
                b_i2(b, 1, 1)
    nc.compile()
    return nc


_CACHE = {}


def kernel(x, weight):
    x = np.asarray(x, dtype=np.float32)
    weight = np.asarray(weight, dtype=np.float32)
    consts = build_constants(weight)
    x_bf = np.ascontiguousarray(
        x.reshape(B, 128, 1024).astype(NPBF))
    if "nc" not in _CACHE:
        _CACHE["nc"] = build_program()
    nc = _CACHE["nc"]
    in_maps = []
    for i in range(N_CORES):
        m = {"x": x_bf[i * B_LOC:(i + 1) * B_LOC]}
        m.update(consts)
        in_maps.append(m)
    res = run_bass_kernel_spmd(nc, in_maps, core_ids=list(range(N_CORES)))
    out = np.concatenate([r["out"] for r in res.results], axis=0)
    return out.reshape(B, 256, 32, 32)


if __name__ == "__main__":
    import jax

    sys.path.insert(0, "/root/problem")
    from reference import setup_inputs, reference

    with jax.default_device(jax.devices("cpu")[0]):
        inputs = setup_inputs()
        inputs = {k_: np.asarray(v) for k_, v in inputs.items()}
        expected = np.asarray(reference(**inputs))
    actual = kernel(**inputs)
    err = np.linalg.norm(actual - expected) / np.linalg.norm(expected)
    print("Relative error:", err)
